# revision 27
# baseline (speedup 1.0000x reference)
"""Trainium2 Bass kernel for the smoothed Preisach hysteresis model.

Math: the reference per-step update
    s' = where(h_t > h_{t-1}, s + (1-s)*sigmoid((h_t-alpha)/temp),
                              s + (-1-s)*sigmoid((beta-h_t)/temp))
is a first-order linear recurrence  s' = a*s + b  with
    g = sigmoid(arg),  a = 1-g,  b = sigma*g
where sigma = +/-1 is the (host-known) step direction and
    arg[n,t] = p_t + alpha_n*q_t + beta_n*r_t
is rank-3 in (n,t). On device: a K=6 fp16 PE matmul (alpha/beta/p split
into hi+lo fp16 pairs for fp32-class accuracy, fp32 PSUM accumulate)
produces arg; ScalarE applies sigmoid (fp16 out); DVE computes
a = 1-g, b = sigma*g and solves the whole T-step recurrence with the
native tensor_tensor_scan (fp32 internal state); the density-weighted
readout is a K=128 fp16 matmul accumulated over the 6 hysteron tiles.

Sharding: hysteron dim N=5151 split across 8 cores (644 each, padded to
6 tiles of 128 partitions; padding carries density 0). Each core outputs
its density-weighted partial sum [1, T]; host reduces across cores and
applies the affine epilogue.
"""

import sys

import numpy as np

sys.path.insert(0, "/opt/trn_rl_repo")

N = 5151
T = 4096
TEMP = 0.01
NCORES = 8
P = 128
TILES = 6                 # ceil(644/128); per-core rows padded to 768
NPC = 644                 # hysterons per core (8*644 = 5152 >= N)
ROWS = TILES * P          # 768
K6 = 6                    # arg matmul contraction: a_hi,a_lo,b_hi,b_lo,1,1
CH = 512                  # matmul chunk along T (one PSUM bank fp32)
NCH = T // CH
ACH = 1024                # ACT chunk (2 PSUM banks per arg tile)
SCH = 2048                # scan chunk along T (TTS chained via initial)
NSCH = T // SCH

_PROG_CACHE = {}


def _build_program(reps=1, loop_n=0, skip=()):
    import contextlib

    import concourse.bass as bass
    import concourse.tile as tile
    from concourse import bacc, mybir

    f32 = mybir.dt.float32
    f16 = mybir.dt.float16
    nc = bacc.Bacc("TRN2", target_bir_lowering=False, debug=False,
                   num_devices=NCORES)

    wt_d = nc.dram_tensor("wt", [K6, ROWS], f16, kind="ExternalInput")
    v_d = nc.dram_tensor("v", [K6, T], f16, kind="ExternalInput")
    sig_d = nc.dram_tensor("sig", [T], f16, kind="ExternalInput")
    dens_d = nc.dram_tensor("dens", [P, TILES], f16, kind="ExternalInput")
    mpart_d = nc.dram_tensor("mpart", [1, T], f32, kind="ExternalOutput")

    wt_ap = wt_d.ap()
    v_ap = v_d.ap()
    sig_ap = sig_d.ap()
    dens_ap = dens_d.ap()
    mpart_ap = mpart_d.ap()

    ts = bass.ts
    Sigmoid = mybir.ActivationFunctionType.Sigmoid
    mult = mybir.AluOpType.mult
    add = mybir.AluOpType.add

    with tile.TileContext(nc) as tc:
        from contextlib import ExitStack
        with ExitStack() as ctx:
            consts = ctx.enter_context(tc.tile_pool(name="consts", bufs=1))
            gpool = ctx.enter_context(tc.tile_pool(name="g", bufs=3))
            apool = ctx.enter_context(tc.tile_pool(name="a", bufs=3))
            bpool = ctx.enter_context(tc.tile_pool(name="b", bufs=3))
            spool = ctx.enter_context(tc.tile_pool(name="s", bufs=TILES))
            mpool = ctx.enter_context(tc.tile_pool(name="m", bufs=1))
            ps_arg = ctx.enter_context(
                tc.tile_pool(name="ps_arg", bufs=3, space="PSUM"))
            ps_m = ctx.enter_context(
                tc.tile_pool(name="ps_m", bufs=2, space="PSUM"))

            wt_sb = consts.tile([K6, ROWS], f16)
            v_sb = consts.tile([K6, T], f16)
            dens_sb = consts.tile([P, TILES], f16)
            sig_bc = consts.tile([P, T], f16)

            nc.sync.dma_start(out=wt_sb[:], in_=wt_ap[:, :])
            nc.sync.dma_start(out=v_sb[:], in_=v_ap[:, :])
            nc.sync.dma_start(out=dens_sb[:], in_=dens_ap[:, :])
            # broadcast sigma row to all 128 partitions via 0-stride DMA,
            # chunked across queues so it doesn't serialize the pipeline
            for j in range(NCH):
                src = bass.AP(tensor=sig_ap.tensor,
                              offset=sig_ap.offset + j * CH,
                              ap=[[0, P], [1, CH]])
                nc.sync.dma_start(out=sig_bc[:, ts(j, CH)], in_=src)

            if loop_n:
                loop_cm = tc.For_i(
                    0, loop_n, 1,
                    hint_engines=(mybir.EngineType.PE,
                                  mybir.EngineType.Activation,
                                  mybir.EngineType.DVE))
            else:
                loop_cm = contextlib.nullcontext()
            with loop_cm:
              for _rep in range(reps):
                s_tiles = []
                for i in range(TILES):
                    s = spool.tile([P, T], f16)
                    g = gpool.tile([P, T], f16)
                    a = apool.tile([P, T], f16)
                    b = bpool.tile([P, T], f16)
                    if "mm" not in skip:
                        for aj in range(T // ACH):
                            arg = ps_arg.tile([P, ACH], f32, tag="arg")
                            for jj in range(ACH // CH):
                                j = aj * (ACH // CH) + jj
                                nc.tensor.matmul(
                                    out=arg[:, ts(jj, CH)],
                                    lhsT=wt_sb[:, ts(i, P)],
                                    rhs=v_sb[:, ts(j, CH)],
                                    start=True, stop=True,
                                )
                            if "act" not in skip:
                                nc.scalar.activation(
                                    out=g[:, ts(aj, ACH)], in_=arg[:],
                                    func=Sigmoid, scale=1.0)
                    if "act" in skip or "mm" in skip:
                        nc.scalar.memzero(g[:])
                    if "ts" not in skip:
                        # a = 1 - g on ScalarE: Copy(g * -1 + 1)
                        nc.scalar.activation(
                            out=a[:], in_=g[:],
                            func=mybir.ActivationFunctionType.Copy,
                            bias=1.0, scale=-1.0)
                    else:
                        nc.vector.memset(a[:], 0.5)
                    if "tt" not in skip:
                        # b = sigma * g on the (otherwise idle) Pool engine
                        nc.gpsimd.tensor_mul(out=b[:], in0=g[:],
                                             in1=sig_bc[:])
                    else:
                        nc.vector.memset(b[:], 0.0)
                    if "scan" not in skip:
                        nc.vector.tensor_tensor_scan(
                            out=s[:], data0=a[:], data1=b[:],
                            initial=-1.0, op0=mult, op1=add,
                        )
                    else:
                        nc.vector.tensor_copy(out=s[:], in_=a[:])
                    s_tiles.append(s)

                m_sb = mpool.tile([1, T], f32)
                if "rd" not in skip:
                    for j in range(NCH):
                        mp = ps_m.tile([1, CH], f32)
                        for i in range(TILES):
                            nc.tensor.matmul(
                                out=mp[:],
                                lhsT=dens_sb[:, i:i + 1],
                                rhs=s_tiles[i][:, ts(j, CH)],
                                start=(i == 0), stop=(i == TILES - 1),
                            )
                        nc.scalar.copy(out=m_sb[:, ts(j, CH)], in_=mp[:])
                else:
                    nc.scalar.memzero(m_sb[:])
                nc.sync.dma_start(out=mpart_ap[:, :], in_=m_sb[:])
    nc.compile()
    return nc


def _split16(x):
    hi = x.astype(np.float16)
    lo = (x - hi.astype(np.float64)).astype(np.float16)
    return hi, lo


def _host_prep(h, mesh_points, raw_density):
    h = np.asarray(h, np.float32)
    mesh = np.asarray(mesh_points, np.float32)
    rd = np.asarray(raw_density, np.float32)
    beta = mesh[:, 0].astype(np.float64)
    alpha = mesh[:, 1].astype(np.float64)

    hprev = np.concatenate([[np.float32(0.0)], h[:-1]])
    up = h > hprev
    sig16 = np.where(up, 1.0, -1.0).astype(np.float16)
    R = np.float64(1.0) / np.float64(np.float32(TEMP))
    h64 = h.astype(np.float64)
    q = np.where(up, -R, 0.0)
    r = np.where(up, 0.0, R)
    p = np.where(up, R * h64, -R * h64)
    p_hi, p_lo = _split16(p)
    V6 = np.stack([q.astype(np.float16), q.astype(np.float16),
                   r.astype(np.float16), r.astype(np.float16),
                   p_hi, p_lo]).astype(np.float16)        # [6, T]

    dens = (1.0 / (1.0 + np.exp(-rd.astype(np.float64))))  # [N] float64

    pad = NCORES * NPC - N   # 1
    alpha_p = np.concatenate([alpha, np.full(pad, 0.5)])
    beta_p = np.concatenate([beta, np.full(pad, 0.5)])
    dens_p = np.concatenate([dens, np.zeros(pad)])

    in_maps = []
    for c in range(NCORES):
        sl = slice(c * NPC, (c + 1) * NPC)
        a_c = np.full(ROWS, 0.5)
        b_c = np.full(ROWS, 0.5)
        d_c = np.zeros(ROWS)
        a_c[:NPC] = alpha_p[sl]
        b_c[:NPC] = beta_p[sl]
        d_c[:NPC] = dens_p[sl]
        ah, al = _split16(a_c)
        bh, bl = _split16(b_c)
        wt = np.stack([ah, al, bh, bl,
                       np.ones(ROWS, np.float16),
                       np.ones(ROWS, np.float16)]).astype(np.float16)
        dens_tiles = d_c.reshape(TILES, P).T.astype(np.float16)  # [P, TILES]
        in_maps.append({
            "wt": wt,
            "v": V6,
            "sig": sig16,
            "dens": dens_tiles,
        })
    return in_maps, dens, h


def kernel(h, mesh_points, raw_density, raw_offset, raw_scale, raw_slope):
    from concourse.bass_utils import run_bass_kernel_spmd

    in_maps, dens, h32 = _host_prep(h, mesh_points, raw_density)

    if "prog" not in _PROG_CACHE:
        _PROG_CACHE["prog"] = _build_program()
    nc = _PROG_CACHE["prog"]

    res = run_bass_kernel_spmd(nc, in_maps, list(range(NCORES)))
    msum = np.zeros(T, np.float64)
    for c in range(NCORES):
        msum += res.results[c]["mpart"].astype(np.float64).reshape(T)

    def sigm(x):
        return 1.0 / (1.0 + np.exp(-np.float64(np.asarray(x, np.float32)[0])))

    offset = -10.0 + 20.0 * sigm(raw_offset)
    scale = 20.0 * sigm(raw_scale)
    slope = -20.0 + 40.0 * sigm(raw_slope)

    m = msum / dens.sum()
    out = scale * m + h32.astype(np.float64) * slope + offset
    return out.astype(np.float32)


# revision 28
# speedup vs baseline: 1.3280x; 1.3280x over previous
"""Trainium2 Bass kernel for the smoothed Preisach hysteresis model.

Math: the reference per-step update
    s' = where(h_t > h_{t-1}, s + (1-s)*sigmoid((h_t-alpha)/temp),
                              s + (-1-s)*sigmoid((beta-h_t)/temp))
is a first-order linear recurrence. In the shifted state u = (s+1)/2:
    u' = a*u + g*M_up,   a = sigmoid(-arg),  g*M_up = sigmoid(arg_b)
where arg[n,t] = p_t + alpha_n*q_t + beta_n*r_t is rank-3 in (n,t)
(K=6 fp16 PE matmul with alpha/beta/p split into hi+lo fp16 pairs for
fp32-class accuracy) and arg_b is a second matmul whose down-step
columns are forced to -60 so sigmoid yields exactly g*M_up. Both scan
operands thus come straight from ScalarE sigmoids; the DVE runs ONLY
the native tensor_tensor_scan (fp32 internal state), which is the
hardware bottleneck (~2.4 ns/element, serial recurrence). The
density-weighted readout is a K=128 fp16 matmul accumulated over the
6 hysteron tiles; m = 2*(d@u) - sum(d) is fixed up on the host.

Sharding: hysteron dim N=5151 split across 8 cores (644 each, padded to
6 tiles of 128 partitions; padding carries density 0). Each core
outputs its readout partials [1, T]; host reduces across cores and
applies the affine epilogue.
"""

import sys

import numpy as np

sys.path.insert(0, "/opt/trn_rl_repo")

N = 5151
T = 4096
TEMP = 0.01
NCORES = 8
P = 128
TILES = 6                 # ceil(644/128); per-core rows padded to 768
NPC = 644                 # hysterons per core (8*644 = 5152 >= N)
ROWS = TILES * P          # 768
K6 = 6                    # arg matmul contraction: a_hi,a_lo,b_hi,b_lo,1,1
CH = 512                  # matmul chunk along T (one PSUM bank fp32)
NCH = T // CH
ACH = 1024                # ACT chunk (2 PSUM banks per arg tile)

_PROG_CACHE = {}


def _build_program(reps=1, loop_n=0, skip=()):
    import contextlib

    import concourse.bass as bass
    import concourse.tile as tile
    from concourse import bacc, mybir

    f32 = mybir.dt.float32
    f16 = mybir.dt.float16
    nc = bacc.Bacc("TRN2", target_bir_lowering=False, debug=False,
                   num_devices=NCORES)

    wt_d = nc.dram_tensor("wt", [K6, ROWS], f16, kind="ExternalInput")
    v_d = nc.dram_tensor("v", [K6, T], f16, kind="ExternalInput")
    vb_d = nc.dram_tensor("vb", [K6, T], f16, kind="ExternalInput")
    dens_d = nc.dram_tensor("dens", [P, TILES], f16, kind="ExternalInput")
    mpart_d = nc.dram_tensor("mpart", [1, T], f32, kind="ExternalOutput")

    wt_ap = wt_d.ap()
    v_ap = v_d.ap()
    vb_ap = vb_d.ap()
    dens_ap = dens_d.ap()
    mpart_ap = mpart_d.ap()

    ts = bass.ts
    Sigmoid = mybir.ActivationFunctionType.Sigmoid
    mult = mybir.AluOpType.mult
    add = mybir.AluOpType.add

    with tile.TileContext(nc) as tc:
        from contextlib import ExitStack
        with ExitStack() as ctx:
            consts = ctx.enter_context(tc.tile_pool(name="consts", bufs=1))
            apool = ctx.enter_context(tc.tile_pool(name="a", bufs=3))
            bpool = ctx.enter_context(tc.tile_pool(name="b", bufs=3))
            spool = ctx.enter_context(tc.tile_pool(name="s", bufs=TILES))
            mpool = ctx.enter_context(tc.tile_pool(name="m", bufs=1))
            ps_arg = ctx.enter_context(
                tc.tile_pool(name="ps_arg", bufs=3, space="PSUM"))
            ps_m = ctx.enter_context(
                tc.tile_pool(name="ps_m", bufs=2, space="PSUM"))

            wt_sb = consts.tile([K6, ROWS], f16)
            v_sb = consts.tile([K6, T], f16)
            vb_sb = consts.tile([K6, T], f16)
            dens_sb = consts.tile([P, TILES], f16)

            nc.sync.dma_start(out=wt_sb[:], in_=wt_ap[:, :])
            nc.sync.dma_start(out=v_sb[:], in_=v_ap[:, :])
            nc.sync.dma_start(out=vb_sb[:], in_=vb_ap[:, :])
            nc.sync.dma_start(out=dens_sb[:], in_=dens_ap[:, :])

            if loop_n:
                loop_cm = tc.For_i(
                    0, loop_n, 1,
                    hint_engines=(mybir.EngineType.PE,
                                  mybir.EngineType.Activation,
                                  mybir.EngineType.DVE))
            else:
                loop_cm = contextlib.nullcontext()
            with loop_cm:
              for _rep in range(reps):
                s_tiles = []
                for i in range(TILES):
                    s = spool.tile([P, T], f16)
                    a = apool.tile([P, T], f16)
                    b = bpool.tile([P, T], f16)
                    for aj in range(T // ACH):
                        arg = ps_arg.tile([P, ACH], f32, tag="arg")
                        argb = ps_arg.tile([P, ACH], f32, tag="arg")
                        for jj in range(ACH // CH):
                            j = aj * (ACH // CH) + jj
                            nc.tensor.matmul(
                                out=arg[:, ts(jj, CH)],
                                lhsT=wt_sb[:, ts(i, P)],
                                rhs=v_sb[:, ts(j, CH)],
                                start=True, stop=True,
                            )
                            nc.tensor.matmul(
                                out=argb[:, ts(jj, CH)],
                                lhsT=wt_sb[:, ts(i, P)],
                                rhs=vb_sb[:, ts(j, CH)],
                                start=True, stop=True,
                            )
                        # a = sigmoid(-arg);  b = g*M_up = sigmoid(arg_b)
                        nc.scalar.activation(
                            out=a[:, ts(aj, ACH)], in_=arg[:],
                            func=Sigmoid, scale=-1.0)
                        nc.scalar.activation(
                            out=b[:, ts(aj, ACH)], in_=argb[:],
                            func=Sigmoid, scale=1.0)
                    if "scan" not in skip:
                        nc.vector.tensor_tensor_scan(
                            out=s[:], data0=a[:], data1=b[:],
                            initial=0.0, op0=mult, op1=add,
                        )
                    else:
                        nc.vector.tensor_copy(out=s[:], in_=a[:])
                    s_tiles.append(s)

                m_sb = mpool.tile([1, T], f32)
                for j in range(NCH):
                    mp = ps_m.tile([1, CH], f32)
                    for i in range(TILES):
                        nc.tensor.matmul(
                            out=mp[:],
                            lhsT=dens_sb[:, i:i + 1],
                            rhs=s_tiles[i][:, ts(j, CH)],
                            start=(i == 0), stop=(i == TILES - 1),
                        )
                    nc.scalar.copy(out=m_sb[:, ts(j, CH)], in_=mp[:])
                nc.sync.dma_start(out=mpart_ap[:, :], in_=m_sb[:])
    nc.compile()
    return nc


def _split16(x):
    hi = x.astype(np.float16)
    lo = (x - hi.astype(np.float64)).astype(np.float16)
    return hi, lo


def _host_prep(h, mesh_points, raw_density):
    h = np.asarray(h, np.float32)
    mesh = np.asarray(mesh_points, np.float32)
    rd = np.asarray(raw_density, np.float32)
    beta = mesh[:, 0].astype(np.float64)
    alpha = mesh[:, 1].astype(np.float64)

    hprev = np.concatenate([[np.float32(0.0)], h[:-1]])
    up = h > hprev
    R = np.float64(1.0) / np.float64(np.float32(TEMP))
    h64 = h.astype(np.float64)
    q = np.where(up, -R, 0.0)
    r = np.where(up, 0.0, R)
    p = np.where(up, R * h64, -R * h64)
    p_hi, p_lo = _split16(p)
    q16 = q.astype(np.float16)
    r16 = r.astype(np.float16)
    V6 = np.stack([q16, q16, r16, r16, p_hi, p_lo]).astype(np.float16)
    # masked variant for data1 = g*M_up: down-step columns forced to -60
    qb = np.where(up, q, 0.0).astype(np.float16)
    rb = np.zeros(T, np.float16)
    pb_hi, pb_lo = _split16(np.where(up, p, -60.0))
    V6b = np.stack([qb, qb, rb, rb, pb_hi, pb_lo]).astype(np.float16)

    dens = (1.0 / (1.0 + np.exp(-rd.astype(np.float64))))  # [N] float64

    pad = NCORES * NPC - N   # 1
    alpha_p = np.concatenate([alpha, np.full(pad, 0.5)])
    beta_p = np.concatenate([beta, np.full(pad, 0.5)])
    dens_p = np.concatenate([dens, np.zeros(pad)])

    in_maps = []
    dens16_sums = []
    for c in range(NCORES):
        sl = slice(c * NPC, (c + 1) * NPC)
        a_c = np.full(ROWS, 0.5)
        b_c = np.full(ROWS, 0.5)
        d_c = np.zeros(ROWS)
        a_c[:NPC] = alpha_p[sl]
        b_c[:NPC] = beta_p[sl]
        d_c[:NPC] = dens_p[sl]
        ah, al = _split16(a_c)
        bh, bl = _split16(b_c)
        wt = np.stack([ah, al, bh, bl,
                       np.ones(ROWS, np.float16),
                       np.ones(ROWS, np.float16)]).astype(np.float16)
        dens16 = d_c.astype(np.float16)
        dens_tiles = dens16.reshape(TILES, P).T  # [P, TILES]
        dens16_sums.append(dens16.astype(np.float64).sum())
        in_maps.append({
            "wt": wt,
            "v": V6,
            "vb": V6b,
            "dens": dens_tiles,
        })
    return in_maps, dens, h, sum(dens16_sums)


def kernel(h, mesh_points, raw_density, raw_offset, raw_scale, raw_slope):
    from concourse.bass_utils import run_bass_kernel_spmd

    in_maps, dens, h32, dens16_sum = _host_prep(h, mesh_points, raw_density)

    if "prog" not in _PROG_CACHE:
        _PROG_CACHE["prog"] = _build_program()
    nc = _PROG_CACHE["prog"]

    res = run_bass_kernel_spmd(nc, in_maps, list(range(NCORES)))
    usum = np.zeros(T, np.float64)
    for c in range(NCORES):
        usum += res.results[c]["mpart"].astype(np.float64).reshape(T)

    def sigm(x):
        return 1.0 / (1.0 + np.exp(-np.float64(np.asarray(x, np.float32)[0])))

    offset = -10.0 + 20.0 * sigm(raw_offset)
    scale = 20.0 * sigm(raw_scale)
    slope = -20.0 + 40.0 * sigm(raw_slope)

    # s = 2u - 1  =>  sum(d*s) = 2*sum(d*u) - sum(d)
    m = (2.0 * usum - dens16_sum) / dens.sum()
    out = scale * m + h32.astype(np.float64) * slope + offset
    return out.astype(np.float32)


# revision 31
# speedup vs baseline: 1.8105x; 1.3633x over previous
"""Trainium2 Bass kernel for the smoothed Preisach hysteresis model.

Math: the reference per-step update
    s' = where(h_t > h_{t-1}, s + (1-s)*sigmoid((h_t-alpha)/temp),
                              s + (-1-s)*sigmoid((beta-h_t)/temp))
is a first-order linear recurrence. In the shifted state u = (s+1)/2:
    u' = a*u + g*M_up,   a = sigmoid(-arg),  g*M_up = sigmoid(arg_b)
where arg[n,t] = p_t + alpha_n*q_t + beta_n*r_t is rank-3 in (n,t)
(K=6 fp16 PE matmul with alpha/beta/p split into hi+lo fp16 pairs for
fp32-class accuracy) and arg_b is a second matmul whose down-step
columns are forced to -60 so sigmoid yields exactly g*M_up. Both scan
operands thus come straight from ScalarE sigmoids; the DVE runs ONLY
the native tensor_tensor_scan (fp32 internal state), which is the
hardware bottleneck (~2.4 ns/element, serial recurrence). The
density-weighted readout is a K=128 fp16 matmul accumulated over the
6 hysteron tiles; m = 2*(d@u) - sum(d) is fixed up on the host.

Sharding: hysteron dim N=5151 split across 8 cores (644 each, padded to
6 tiles of 128 partitions; padding carries density 0). Each core
outputs its readout partials [1, T]; host reduces across cores and
applies the affine epilogue.
"""

import sys

import numpy as np

sys.path.insert(0, "/opt/trn_rl_repo")

N = 5151
T = 4096
TEMP = 0.01
NCORES = 8
P = 128
TILES = 6                 # ceil(644/128); per-core rows padded to 768
NPC = 644                 # hysterons per core (8*644 = 5152 >= N)
ROWS = TILES * P          # 768
K6 = 6                    # arg matmul contraction: a_hi,a_lo,b_hi,b_lo,1,1
CH = 512                  # matmul chunk along T (one PSUM bank fp32)
NCH = T // CH
ACH = 1024                # ACT chunk (2 PSUM banks per arg tile)
SCH = 2048                # scan chunk along T (TTS chained via initial)
NSCH = T // SCH

_PROG_CACHE = {}


def _build_program(reps=1, loop_n=0, skip=()):
    import contextlib

    import concourse.bass as bass
    import concourse.tile as tile
    from concourse import bacc, mybir

    f32 = mybir.dt.float32
    f16 = mybir.dt.float16
    nc = bacc.Bacc("TRN2", target_bir_lowering=False, debug=False,
                   num_devices=NCORES)

    wt_d = nc.dram_tensor("wt", [K6, ROWS], f16, kind="ExternalInput")
    v_d = nc.dram_tensor("v", [K6, T], f16, kind="ExternalInput")
    vb_d = nc.dram_tensor("vb", [K6, T], f16, kind="ExternalInput")
    dens_d = nc.dram_tensor("dens", [P, TILES], f16, kind="ExternalInput")
    mpart_d = nc.dram_tensor("mpart", [1, T], f32, kind="ExternalOutput")

    wt_ap = wt_d.ap()
    v_ap = v_d.ap()
    vb_ap = vb_d.ap()
    dens_ap = dens_d.ap()
    mpart_ap = mpart_d.ap()

    ts = bass.ts
    Sigmoid = mybir.ActivationFunctionType.Sigmoid
    mult = mybir.AluOpType.mult
    add = mybir.AluOpType.add

    with tile.TileContext(nc) as tc:
        from contextlib import ExitStack
        with ExitStack() as ctx:
            consts = ctx.enter_context(tc.tile_pool(name="consts", bufs=1))
            apool = ctx.enter_context(tc.tile_pool(name="a", bufs=3))
            bpool = ctx.enter_context(tc.tile_pool(name="b", bufs=3))
            spool = ctx.enter_context(tc.tile_pool(name="s", bufs=TILES))
            mpool = ctx.enter_context(tc.tile_pool(name="m", bufs=1))
            ps_arg = ctx.enter_context(
                tc.tile_pool(name="ps_arg", bufs=3, space="PSUM"))
            ps_m = ctx.enter_context(
                tc.tile_pool(name="ps_m", bufs=2, space="PSUM"))

            wt_sb = consts.tile([K6, ROWS], f16)
            v_sb = consts.tile([K6, T], f16)
            vb_sb = consts.tile([K6, T], f16)
            dens_sb = consts.tile([P, TILES], f16)

            nc.sync.dma_start(out=wt_sb[:], in_=wt_ap[:, :])
            nc.sync.dma_start(out=v_sb[:], in_=v_ap[:, :])
            nc.sync.dma_start(out=vb_sb[:], in_=vb_ap[:, :])
            nc.sync.dma_start(out=dens_sb[:], in_=dens_ap[:, :])

            if loop_n:
                loop_cm = tc.For_i(
                    0, loop_n, 1,
                    hint_engines=(mybir.EngineType.PE,
                                  mybir.EngineType.Activation,
                                  mybir.EngineType.DVE))
            else:
                loop_cm = contextlib.nullcontext()
            with loop_cm:
              for _rep in range(reps):
                s_tiles = []
                for i in range(TILES):
                    s = spool.tile([P, T], f16)
                    for c in range(NSCH):
                        a = apool.tile([P, SCH], f16)
                        b = bpool.tile([P, SCH], f16)
                        for aj in range(SCH // ACH):
                            arg = ps_arg.tile([P, ACH], f32, tag="arg")
                            argb = ps_arg.tile([P, ACH], f32, tag="arg")
                            for jj in range(ACH // CH):
                                j = (c * SCH + aj * ACH) // CH + jj
                                nc.tensor.matmul(
                                    out=arg[:, ts(jj, CH)],
                                    lhsT=wt_sb[:, ts(i, P)],
                                    rhs=v_sb[:, ts(j, CH)],
                                    start=True, stop=True,
                                )
                                nc.tensor.matmul(
                                    out=argb[:, ts(jj, CH)],
                                    lhsT=wt_sb[:, ts(i, P)],
                                    rhs=vb_sb[:, ts(j, CH)],
                                    start=True, stop=True,
                                )
                            # a = sigmoid(-arg);  b = g*M_up = sigmoid(argb)
                            nc.scalar.activation(
                                out=a[:, ts(aj, ACH)], in_=arg[:],
                                func=Sigmoid, scale=-1.0)
                            nc.scalar.activation(
                                out=b[:, ts(aj, ACH)], in_=argb[:],
                                func=Sigmoid, scale=1.0)
                        if "scan" not in skip:
                            init = (0.0 if c == 0
                                    else s[:, c * SCH - 1:c * SCH])
                            nc.vector.tensor_tensor_scan(
                                out=s[:, ts(c, SCH)], data0=a[:],
                                data1=b[:],
                                initial=init, op0=mult, op1=add,
                            )
                        else:
                            nc.vector.tensor_copy(out=s[:, ts(c, SCH)],
                                                  in_=a[:])
                    s_tiles.append(s)

                m_sb = mpool.tile([1, T], f32)
                for j in range(NCH):
                    mp = ps_m.tile([1, CH], f32)
                    for i in range(TILES):
                        nc.tensor.matmul(
                            out=mp[:],
                            lhsT=dens_sb[:, i:i + 1],
                            rhs=s_tiles[i][:, ts(j, CH)],
                            start=(i == 0), stop=(i == TILES - 1),
                        )
                    nc.scalar.copy(out=m_sb[:, ts(j, CH)], in_=mp[:])
                nc.sync.dma_start(out=mpart_ap[:, :], in_=m_sb[:])
    nc.compile()
    return nc


def _split16(x):
    hi = x.astype(np.float16)
    lo = (x - hi.astype(np.float64)).astype(np.float16)
    return hi, lo


def _host_prep(h, mesh_points, raw_density):
    h = np.asarray(h, np.float32)
    mesh = np.asarray(mesh_points, np.float32)
    rd = np.asarray(raw_density, np.float32)
    beta = mesh[:, 0].astype(np.float64)
    alpha = mesh[:, 1].astype(np.float64)

    hprev = np.concatenate([[np.float32(0.0)], h[:-1]])
    up = h > hprev
    R = np.float64(1.0) / np.float64(np.float32(TEMP))
    h64 = h.astype(np.float64)
    q = np.where(up, -R, 0.0)
    r = np.where(up, 0.0, R)
    p = np.where(up, R * h64, -R * h64)
    p_hi, p_lo = _split16(p)
    q16 = q.astype(np.float16)
    r16 = r.astype(np.float16)
    V6 = np.stack([q16, q16, r16, r16, p_hi, p_lo]).astype(np.float16)
    # masked variant for data1 = g*M_up: down-step columns forced to -60
    qb = np.where(up, q, 0.0).astype(np.float16)
    rb = np.zeros(T, np.float16)
    pb_hi, pb_lo = _split16(np.where(up, p, -60.0))
    V6b = np.stack([qb, qb, rb, rb, pb_hi, pb_lo]).astype(np.float16)

    dens = (1.0 / (1.0 + np.exp(-rd.astype(np.float64))))  # [N] float64

    pad = NCORES * NPC - N   # 1
    alpha_p = np.concatenate([alpha, np.full(pad, 0.5)])
    beta_p = np.concatenate([beta, np.full(pad, 0.5)])
    dens_p = np.concatenate([dens, np.zeros(pad)])

    in_maps = []
    dens16_sums = []
    for c in range(NCORES):
        sl = slice(c * NPC, (c + 1) * NPC)
        a_c = np.full(ROWS, 0.5)
        b_c = np.full(ROWS, 0.5)
        d_c = np.zeros(ROWS)
        a_c[:NPC] = alpha_p[sl]
        b_c[:NPC] = beta_p[sl]
        d_c[:NPC] = dens_p[sl]
        ah, al = _split16(a_c)
        bh, bl = _split16(b_c)
        wt = np.stack([ah, al, bh, bl,
                       np.ones(ROWS, np.float16),
                       np.ones(ROWS, np.float16)]).astype(np.float16)
        dens16 = d_c.astype(np.float16)
        dens_tiles = dens16.reshape(TILES, P).T  # [P, TILES]
        dens16_sums.append(dens16.astype(np.float64).sum())
        in_maps.append({
            "wt": wt,
            "v": V6,
            "vb": V6b,
            "dens": dens_tiles,
        })
    return in_maps, dens, h, sum(dens16_sums)


def kernel(h, mesh_points, raw_density, raw_offset, raw_scale, raw_slope):
    from concourse.bass_utils import run_bass_kernel_spmd

    in_maps, dens, h32, dens16_sum = _host_prep(h, mesh_points, raw_density)

    if "prog" not in _PROG_CACHE:
        _PROG_CACHE["prog"] = _build_program()
    nc = _PROG_CACHE["prog"]

    res = run_bass_kernel_spmd(nc, in_maps, list(range(NCORES)))
    usum = np.zeros(T, np.float64)
    for c in range(NCORES):
        usum += res.results[c]["mpart"].astype(np.float64).reshape(T)

    def sigm(x):
        return 1.0 / (1.0 + np.exp(-np.float64(np.asarray(x, np.float32)[0])))

    offset = -10.0 + 20.0 * sigm(raw_offset)
    scale = 20.0 * sigm(raw_scale)
    slope = -20.0 + 40.0 * sigm(raw_slope)

    # s = 2u - 1  =>  sum(d*s) = 2*sum(d*u) - sum(d)
    m = (2.0 * usum - dens16_sum) / dens.sum()
    out = scale * m + h32.astype(np.float64) * slope + offset
    return out.astype(np.float32)
